# revision 12
# baseline (speedup 1.0000x reference)
"""Bass/Trainium2 kernel for CausalSelfAttention (B=8, T=1024, C=768, H=12).

Sharding: data-parallel over batch. 8 cores, one batch element per core.
No collectives. Each core runs an identical SPMD program on its own slice.

v2 changes over baseline:
  - HAM warm-up: dummy matmuls on a memset scratch tile during the DMA load
    ramp so real matmuls start at 2.4 GHz.
  - DMA loads split/reordered so the first projection tiles arrive ASAP.
  - Per-kt score psum is one [128,1024] tile holding both heads of a pair;
    softmax exp is ONE scalar ACTIVATE over both heads (halves the per-op
    352-cycle overhead and the ACT instruction count).
  - Software-pipelined kt loop: scores(kt+1) issue before PV(kt), so the
    exp latency hides under PE work instead of stalling the PE queue.
  - Normalization: reciprocal on the [1,1024] sums row first, then a gpsimd
    partition_broadcast (no PE broadcast matmuls, far less DVE work).
  - Diagonal causal mask applied to both heads with one DVE op.
  - Phase E of q-tiles 0..3 interleaved into the last pair's second half.

Per-core layouts (host-prepared):
  xT   [768, 1024] bf16   x[b].T
  wqk  [768, 1536] bf16   W_attn[:, :1536], Q columns pre-scaled by 1/sqrt(64)
  wv   [768, 768]  bf16   W_attn[:, 1536:]
  wp   [768, 768]  bf16   W_proj
  bqk  [128, 12]  f32     b_attn[:1536] per-tile columns (Q part pre-scaled)
  bv   [128, 768] f32     b_attn[1536:] broadcast over partitions
  bp   [128, 768] f32     b_proj broadcast over partitions
  qm   [128, 8]   f32     query_mask as per-partition columns per q-tile
  dm   [128, 2, 8, 128] bf16 diagonal-block masks, duplicated per head pair
Output: y [1024, 768] f32 per core.
"""

import sys

if "/opt/trn_rl_repo" not in sys.path:
    sys.path.insert(0, "/opt/trn_rl_repo")

import numpy as np
import ml_dtypes

import concourse.bass as bass
import concourse.bacc as bacc
import concourse.mybir as mybir
import concourse.tile as tile
from concourse.bass import ts, ds

BF16 = mybir.dt.bfloat16
F32 = mybir.dt.float32
AF = mybir.ActivationFunctionType
ALU = mybir.AluOpType
BF16NP = ml_dtypes.bfloat16

T, C, H, HD = 1024, 768, 12, 64
NCORES = 8

_CACHE = {}


def build_program():
    """Build the single-core SPMD Bass program."""
    nc = bacc.Bacc("TRN2", target_bir_lowering=False, debug=False)

    xT_d = nc.dram_tensor("xT", [128, 6 * T], BF16, kind="ExternalInput")
    wqk_d = nc.dram_tensor("wqk", [128, 12 * 768], BF16, kind="ExternalInput")
    wv_d = nc.dram_tensor("wv", [128, 6 * C], BF16, kind="ExternalInput")
    wp_d = nc.dram_tensor("wp", [128, 6 * C], BF16, kind="ExternalInput")
    bqk_d = nc.dram_tensor("bqk", [128, 12], F32, kind="ExternalInput")
    bv_d = nc.dram_tensor("bv", [128, C], F32, kind="ExternalInput")
    bp_d = nc.dram_tensor("bp", [128, C], F32, kind="ExternalInput")
    qm_d = nc.dram_tensor("qm", [128, 8], F32, kind="ExternalInput")
    dm_d = nc.dram_tensor("dm", [128, 2, 8, 128], BF16, kind="ExternalInput")
    y_d = nc.dram_tensor("y", [T, C], F32, kind="ExternalOutput")

    with tile.TileContext(nc) as tc:
        with (
            tc.tile_pool(name="const", bufs=1) as cp,
            tc.tile_pool(name="ptp", bufs=8) as ptp,
            tc.tile_pool(name="recp", bufs=2) as recp,
            tc.tile_pool(name="bcp", bufs=2) as bcp,
            tc.tile_pool(name="otxp", bufs=3) as otxp,
            tc.tile_pool(name="ysb", bufs=3) as ysbp,
            tc.tile_pool(name="ps_a", bufs=2, space="PSUM") as ps_a,
            tc.tile_pool(name="ps_b", bufs=1, space="PSUM") as ps_b,
            tc.tile_pool(name="ps_o", bufs=3, space="PSUM") as ps_o,
        ):
            # ---------------- persistent SBUF tensors ----------------
            xT_sb = cp.tile([128, 6, T], BF16, name="xT_sb")
            wqk_sb = cp.tile([128, 12, 6, 128], BF16, name="wqk_sb")
            wv_sb = cp.tile([128, 6, C], BF16, name="wv_sb")
            wp_sb = cp.tile([128, 6, C], BF16, name="wp_sb")
            bqk_sb = cp.tile([128, 12], F32, name="bqk_sb")
            bv_sb = cp.tile([128, C], F32, name="bv_sb")
            bp_sb = cp.tile([128, C], F32, name="bp_sb")
            qm_sb = cp.tile([128, 8], F32, name="qm_sb")
            dm_sb = cp.tile([128, 2, 8, 128], BF16, name="dm_sb")
            scr_sb = cp.tile([128, 512], BF16, name="scr_sb")
            qk_sb = [cp.tile([128, T], BF16, name=f"qk{m}") for m in range(12)]
            v_sb = [cp.tile([128, 12 * 128], BF16, name=f"v{t}") for t in range(8)]
            ot_sb = cp.tile([128, 6, T], BF16, name="ot_sb")

            # ---------------- warm-up + loads ----------------
            # memset scratch, then dummy matmuls keep the PE HAM busy during
            # the DMA ramp so real matmuls start at 2.4 GHz
            nc.gpsimd.memset(scr_sb[:], 0.0)

            # all weight/activation DRAM layouts are partition-major
            # contiguous: each DMA is 128 x multi-KB contiguous lines
            xT_ap = xT_d[:, :].rearrange("p (k t) -> p k t", t=T)
            wqk_ap = wqk_d[:, :].rearrange("p (m k c) -> p m k c", m=12, k=6)
            wv_ap = wv_d[:, :].rearrange("p (k c) -> p k c", k=6)
            wp_ap = wp_d[:, :].rearrange("p (k c) -> p k c", k=6)
            # critical path first: bias cols, Q m=0, xT, K m=6
            nc.sync.dma_start(bqk_sb[:], bqk_d[:, :])
            nc.sync.dma_start(wqk_sb[:, 0:1, :, :], wqk_ap[:, 0:1, :, :])
            nc.sync.dma_start(wqk_sb[:, 6:7, :, :], wqk_ap[:, 6:7, :, :])
            nc.sync.dma_start(xT_sb[:, :, :], xT_ap[:, :, :])
            nc.sync.dma_start(wqk_sb[:, 1:6, :, :], wqk_ap[:, 1:6, :, :])
            nc.sync.dma_start(wqk_sb[:, 7:12, :, :], wqk_ap[:, 7:12, :, :])
            nc.scalar.dma_start(wv_sb[:, :, :], wv_ap[:, :, :])
            nc.scalar.dma_start(bv_sb[:], bv_d[:, :])
            nc.gpsimd.dma_start(dm_sb[:], dm_d[:, :, :, :])
            nc.gpsimd.dma_start(qm_sb[:], qm_d[:, :])
            nc.gpsimd.dma_start(wp_sb[:, :, :], wp_ap[:, :, :])
            nc.gpsimd.dma_start(bp_sb[:], bp_d[:, :])
            # V head block = [ones, 63 zeros, V(64)]: softmax sums land in
            # psO row 0 (recip/broadcast need base partition 0) and O lands at
            # rows 64:128 (DVE ops only accept base partitions 0/64); the full
            # 128-col stationary also enables FWL on the PV weight loads
            for t in range(8):
                nc.gpsimd.memset(v_sb[t][:], 0.0)
                nc.gpsimd.memset(
                    v_sb[t].rearrange("p (h d) -> p h d", d=128)[:, :, 0:1], 1.0
                )

            warm_ps = ps_a.tile([128, 1024], F32, name="warm_ps", tag="a")
            for _ in range(10):
                nc.tensor.matmul(
                    warm_ps[:, 0:512],
                    scr_sb[:, 0:128],
                    scr_sb[:, :],
                    start=True,
                    stop=True,
                )

            # ---------------- phase B helper: one qkT m-tile ----------------
            # emit_qk_mms emits the 12 matmuls (interleavable with phase D);
            # emit_qk_fin emits the psum->sbuf bias+cast
            def emit_qk_half(m, j):
                pB = ps_b.tile([128, 512], F32, name="pB", tag="b")
                for k in range(6):
                    nc.tensor.matmul(
                        pB[:],
                        wqk_sb[:, m, k, :],
                        xT_sb[:, k, ts(j, 512)],
                        start=(k == 0),
                        stop=(k == 5),
                    )
                # cast+bias on vector: scalar engine is reserved for exp
                nc.vector.tensor_scalar_add(
                    qk_sb[m][:, ts(j, 512)], pB[:], bqk_sb[:, m : m + 1]
                )

            def emit_qk(m):
                for j in range(2):
                    emit_qk_half(m, j)

            emit_qk(0)
            emit_qk(6)

            # ---------------- phase C: V = x @ W_v + bv ----------------
            for t in range(8):
                psv = ps_a.tile([128, 1024], F32, name="psv", tag="a")
                for c0, cw in ((0, 512), (512, 256)):
                    for k in range(6):
                        nc.tensor.matmul(
                            psv[:, ds(c0, cw)],
                            xT_sb[:, k, ts(t, 128)],
                            wv_sb[:, k, ds(c0, cw)],
                            start=(k == 0),
                            stop=(k == 5),
                        )
                nc.vector.tensor_add(
                    v_sb[t].rearrange("p (h d) -> p h d", d=128)[:, :, 64:128],
                    psv[:, 0:768].rearrange("p (h d) -> p h d", d=64),
                    bv_sb[:, :].rearrange("p (h d) -> p h d", d=64),
                )

            # ---------------- phase D+B interleaved per head-pair ----------------
            # per (pair, sbi): kt-pipelined scores/exp/PV; next pair's qk
            # matmul chunks and (for the last pair) phase-E chunks are
            # interleaved between kt steps to keep the PE fed while exp runs.
            def emit_E(qt):
                psy = ps_a.tile([128, 1024], F32, name="psy", tag="a")
                for c0, cw in ((0, 512), (512, 256)):
                    for k in range(6):
                        nc.tensor.matmul(
                            psy[:, ds(c0, cw)],
                            ot_sb[:, k, ts(qt, 128)],
                            wp_sb[:, k, ds(c0, cw)],
                            start=(k == 0),
                            stop=(k == 5),
                        )
                ysb = ysbp.tile([128, C], F32, name="ysb", tag="ysb")
                nc.vector.scalar_tensor_tensor(
                    out=ysb[:],
                    in0=psy[:, 0:768],
                    scalar=qm_sb[:, qt : qt + 1],
                    in1=bp_sb[:],
                    op0=ALU.mult,
                    op1=ALU.add,
                )
                nc.sync.dma_start(y_d[ts(qt, 128), :], ysb[:])

            for pr in range(6):
                # work items to interleave between kt steps of this pair:
                # each emits a chunk of B (next pair's qk halves) or E (for
                # the last pair), spread evenly across the pair's 12 kt steps
                filler = []
                if pr < 5:
                    for m in (pr + 1, 7 + pr):
                        for j in range(2):
                            filler.append((emit_qk_half, m, j))
                elif pr == 5:
                    for qt in range(4):
                        filler.append((emit_E, qt))

                fidx = 0
                step = 0

                def run_filler(sched):
                    nonlocal fidx
                    if fidx < len(filler) and step >= sched[min(fidx, len(sched) - 1)]:
                        f = filler[fidx]
                        f[0](*f[1:])
                        fidx += 1

                # B fillers: spread over all 12 steps; E fillers (pr=5) can
                # only run after pr5-sbi0's normalization -> steps 5..11
                sched = [1, 4, 7, 10] if pr < 5 else [5, 7, 9, 11]

                hs = (2 * pr, 2 * pr + 1)
                for sbi in range(2):
                    q0 = sbi * 512
                    nkt = 4 + 4 * sbi
                    psO = {
                        h: ps_o.tile([128, 512], F32, name="psO", tag="o")
                        for h in hs
                    }
                    pend = []  # software pipeline: delayed PV emissions

                    def emit_pv(kt, dc, w, ptt):
                        for hi, h in enumerate(hs):
                            nc.tensor.matmul(
                                psO[h][:, ds(dc, w)],
                                v_sb[kt][:, h * 128 : h * 128 + 128],
                                ptt[:, hi, ds(dc, w)],
                                start=(kt == 0),
                                stop=(kt == nkt - 1),
                                skip_group_check=True,
                            )

                    for kt in range(nkt):
                        dc = max(0, kt * 128 - q0)
                        w = 512 - dc
                        sps = ps_a.tile([128, 1024], F32, name="sps", tag="a")
                        for hi, h in enumerate(hs):
                            qp = (h % 2) * 64
                            nc.tensor.matmul(
                                sps[:, ds(hi * 512 + dc, w)],
                                qk_sb[6 + h // 2][qp : qp + 64, ts(kt, 128)],
                                qk_sb[h // 2][qp : qp + 64, ds(q0 + dc, w)],
                                start=True,
                                stop=True,
                            )
                        ptt = ptp.tile([128, 2, 512], BF16, name="ptt", tag="ptt")
                        nc.scalar.activation(
                            ptt[:, :, ds(dc, w)],
                            sps.rearrange("p (h q) -> p h q", h=2)[:, :, ds(dc, w)],
                            AF.Exp,
                        )
                        if kt * 128 >= q0:
                            nc.vector.tensor_mul(
                                ptt[:, :, ds(dc, 128)],
                                ptt[:, :, ds(dc, 128)],
                                dm_sb[:, :, kt, :],
                            )
                        pend.append((kt, dc, w, ptt))
                        run_filler(sched)
                        if len(pend) > 1:
                            emit_pv(*pend.pop(0))
                        step += 1
                    emit_pv(*pend.pop(0))

                    # normalize: recip the sums rows (psO row 0), broadcast to
                    # lanes 1..64, multiply, DMA-stage into ot_sb
                    sums = recp.tile([128, 1024], F32, name="sums", tag="sums")
                    bcb = bcp.tile([128, 1024], F32, name="bcb", tag="bcb")
                    for hi, h in enumerate(hs):
                        nc.vector.reciprocal_approx_fast(
                            sums[0:1, ds(hi * 512, 512)], psO[h][0:1, :]
                        )
                    nc.gpsimd.partition_broadcast(bcb[0:128, :], sums[0:1, :])
                    for hi, h in enumerate(hs):
                        j = h // 2
                        r0 = (h % 2) * 64
                        otx = otxp.tile([128, 512], BF16, name="otx", tag="otx")
                        nc.vector.tensor_mul(
                            otx[64:128, :],
                            psO[h][64:128, :],
                            bcb[64:128, ds(hi * 512, 512)],
                        )
                        nc.sync.dma_start(
                            ot_sb[r0 : r0 + 64, j, ds(q0, 512)], otx[64:128, :]
                        )
                    if sbi == 1:
                        # drain any leftover filler
                        while fidx < len(filler):
                            f = filler[fidx]
                            f[0](*f[1:])
                            fidx += 1

            # ---------------- phase E: remaining q-tiles ----------------
            for qt in range(4, 8):
                emit_E(qt)

    nc.compile()
    return nc


def _get_nc():
    if "nc" not in _CACHE:
        _CACHE["nc"] = build_program()
    return _CACHE["nc"]


def prep_core_inputs(x, mask, query_mask, W_attn, b_attn, W_proj, b_proj):
    """Host-side prep. Returns per_core list of input dicts per batch element."""
    scale = 1.0 / np.sqrt(HD)
    W_s = np.asarray(W_attn, np.float32).copy()
    W_s[:, :C] *= scale
    b_s = np.asarray(b_attn, np.float32).copy()
    b_s[:C] *= scale

    def pmaj(w):
        # [768, N] -> [128, 6*N] with [p, k*N + c] = w[k*128+p, c]
        n = w.shape[1]
        return np.ascontiguousarray(
            w.reshape(6, 128, n).transpose(1, 0, 2).reshape(128, 6 * n)
        )

    wqk_f = W_s[:, : 2 * C].astype(BF16NP)  # [768, 1536]
    # m-major: [p, m, k, c] = wqk[k*128+p, m*128+c]
    wqk_p = np.ascontiguousarray(
        wqk_f.reshape(6, 128, 12, 128).transpose(1, 2, 0, 3).reshape(128, -1)
    )
    shared = {
        "wqk": wqk_p,
        "wv": pmaj(W_s[:, 2 * C :].astype(BF16NP)),
        "wp": pmaj(np.asarray(W_proj, np.float32).astype(BF16NP)),
        "bqk": np.ascontiguousarray(b_s[: 2 * C].reshape(12, 128).T),
        "bv": np.ascontiguousarray(
            np.broadcast_to(b_s[2 * C :], (128, C))
        ).astype(np.float32),
        "bp": np.ascontiguousarray(
            np.broadcast_to(np.asarray(b_proj, np.float32), (128, C))
        ),
    }

    per_core = []
    for b in range(NCORES):
        xTf = np.ascontiguousarray(np.asarray(x[b], np.float32).T).astype(BF16NP)
        xT = np.ascontiguousarray(
            xTf.reshape(6, 128, T).transpose(1, 0, 2).reshape(128, 6 * T)
        )
        qm = np.ascontiguousarray(
            np.asarray(query_mask[b, 0, :, 0], np.float32).reshape(8, 128).T
        )
        mb = np.asarray(mask[b, 0])  # [T, T] bool
        blocks = [
            mb[qi * 128 : (qi + 1) * 128, qi * 128 : (qi + 1) * 128].T
            for qi in range(8)
        ]
        dm1 = np.stack(blocks, axis=1).astype(BF16NP)  # [128, 8, 128]
        dm = np.ascontiguousarray(
            np.broadcast_to(dm1[:, None, :, :], (128, 2, 8, 128))
        )
        per_core.append({"xT": xT, "qm": qm, "dm": dm, **shared})
    return per_core


def run_on_cores(inputs, trace=False, **kw):
    from concourse.bass_utils import run_bass_kernel_spmd

    nc = _get_nc()
    in_maps = prep_core_inputs(**inputs)
    res = run_bass_kernel_spmd(
        nc, in_maps, core_ids=list(range(NCORES)), trace=trace, **kw
    )
    out = np.stack([res.results[b]["y"] for b in range(NCORES)], axis=0)
    return out.astype(np.float32), res


def kernel(**inputs) -> np.ndarray:
    out, _ = run_on_cores(inputs, trace=False)
    return out
